# revision 8
# baseline (speedup 1.0000x reference)
"""Trainium2 kernel for nn_BranchModel_9680856285960 (moe_routing).

Math: the reference scatters per-branch sparse weights into dense
(n_br, n_out, n_in) tensors, einsums against x, then takes a context-
gated masked sum over branches followed by relu.  Because the mask-
weighted branch sum commutes with the contraction over input features,
the whole model collapses to a 3-layer dense MLP

    out = relu(relu(x @ Weff1.T) @ Weff2.T) @ W3 + b3

where  Weff_l[o, i] = sum_{r,k} masks_l[ctx, r, o] * w_l[r, o, k]
                                * [idx_l[r, o, k] == i].

The effective-weight fold (a scatter-add over 5.6M index/value pairs) is
data-dependent element-granular addressing, which Trainium2 has no fast
engine for; it is done once on the host, and the device runs the dense
pipeline.

Two exact reductions beyond the baseline:
  * Dead-unit pruning: with 80% gate sparsity, ~10.7% of hidden units
    have ALL branches masked -> their Weff row is identically zero and
    the unit contributes nothing.  Those rows/columns are dropped
    exactly (h = relu(0) = 0), shrinking both layers' weights ~19%.
  * Weights are host-packed partition-major, grouped by output-column
    chunk, so each chunk is ONE ~1-1.6MB DMA with >=4KB-per-partition
    descriptors (HBM line rate), and compute consumes chunks as they
    land instead of waiting on dozens of small semaphore-chained DMAs.

Sharding: data-parallel over batch (8 cores x 128 rows), effective
weights replicated per core, fp16 on the wire, fp32 PSUM accumulation.
No collectives.
"""

import os
import sys
import numpy as np

for _p in ("/opt/trn_rl_repo",):
    if os.path.isdir(_p) and _p not in sys.path:
        sys.path.append(_p)

from contextlib import ExitStack

from concourse import bass, mybir
import concourse.bacc as bacc
import concourse.tile as tile
from concourse.bass_utils import run_bass_kernel_spmd
from concourse.masks import make_identity

F32 = mybir.dt.float32
F16 = mybir.dt.float16

BATCH, NIN, NH, NOUT = 1024, 784, 2000, 10
NCORES = 8
BS = BATCH // NCORES            # 128 batch rows per core
P = 128
KT1 = 7                         # L1 contraction tiles (784 -> 7x128 padded)

# Exposed for the test harness: the BassKernelResults of the last run.
LAST_RESULT = None
_CACHE = {}


def _chunks(n_cols, first):
    """Output-column chunks: small first chunk (starts PE early), the
    rest 512 wide (one PSUM bank), all multiples of 128."""
    assert n_cols % 128 == 0
    out = []
    first = min(first, n_cols)
    out.append((0, first))
    off = first
    while off < n_cols:
        w = min(512, n_cols - off)
        out.append((off, w))
        off += w
    return out


def _build_weff(w, idx, mask_row, n_in):
    """Fold masks + branch sum into a dense effective weight matrix.

    Weff[o, i] = sum_{r,k} mask_row[r, o] * w[r, o, k] * [idx[r, o, k] == i]
    """
    n_br, n_out, npb = w.shape
    acc = np.zeros(n_out * n_in, np.float64)
    base = (np.arange(n_out, dtype=np.int64) * n_in)[:, None]
    for r in range(n_br):
        flat = (base + idx[r].astype(np.int64)).ravel()
        vals = (w[r].astype(np.float64) * mask_row[r].astype(np.float64)[:, None]).ravel()
        acc += np.bincount(flat, weights=vals, minlength=n_out * n_in)
    return acc.reshape(n_out, n_in).astype(np.float32)


def _pack_chunks(wt, kt, chunks):
    """Pack wt (n_in_padded=kt*128 rows, n_cols) into the on-wire layout:
    flat[p, chunk-major: (c, t, col)] = wt[t*128+p, c0+col], so one chunk
    is per-partition contiguous (kt * w * 2 bytes)."""
    n_cols = wt.shape[1]
    total = kt * n_cols
    out = np.zeros((P, total), np.float16)
    pos = 0
    for (c0, w) in chunks:
        blk = wt[:, c0:c0 + w].reshape(kt, P, w)      # [t, p, col]
        out[:, pos:pos + kt * w] = blk.transpose(1, 0, 2).reshape(P, kt * w)
        pos += kt * w
    return out


def _mlp_body(tc, n1t, n2t, xT, w1pk, w2pk, w3p, b3r, out):
    nc = tc.nc
    n1, n2 = n1t * P, n2t * P
    ch1 = _chunks(n1, 128)
    ch2 = _chunks(n2, 512)

    with ExitStack() as ctx:
        const = ctx.enter_context(tc.tile_pool(name="const", bufs=1))
        wp = ctx.enter_context(tc.tile_pool(name="wslab", bufs=1))
        act = ctx.enter_context(tc.tile_pool(name="act", bufs=1))
        pacc = ctx.enter_context(tc.tile_pool(name="pacc", bufs=1, space="PSUM"))
        ptr = ctx.enter_context(tc.tile_pool(name="ptr", bufs=1, space="PSUM"))

        ident = const.tile([P, P], F16, tag="ident")
        make_identity(nc, ident[:])

        # x on sync, first w1 chunk on scalar: both in flight immediately.
        xbig = const.tile([P, KT1, BS], F16, tag="xbig")
        nc.sync.dma_start(out=xbig[:], in_=xT)
        xts = [xbig[:, t, :] for t in range(KT1)]

        b3t = const.tile([NOUT, 1], F32, tag="b3")
        nc.gpsimd.dma_start(out=b3t[:], in_=b3r)
        w3t = const.tile([P, n2t, NOUT], F16, tag="w3")
        nc.gpsimd.dma_start(out=w3t[:], in_=w3p)

        # Weight chunk slabs: each chunk is ONE large per-partition-
        # contiguous DMA.  Ring assignment balances bytes and keeps each
        # ring's FIFO in consumption order.
        n_ch1 = len(ch1)
        ring_of = {}
        for i in range(n_ch1):
            ring_of[("w1", i)] = [nc.scalar, nc.sync][i % 2]
        for i in range(len(ch2)):
            ring_of[("w2", i)] = [nc.scalar, nc.sync][(i + n_ch1) % 2]

        w1s, pos = [], 0
        for i, (c0, w) in enumerate(ch1):
            slab = wp.tile([P, KT1, w], F16, name=f"w1s{i}", tag=f"w1s{i}")
            ring_of[("w1", i)].dma_start(out=slab[:], in_=w1pk[:, pos:pos + KT1 * w])
            w1s.append(slab)
            pos += KT1 * w
        w2s, pos = [], 0
        for i, (c0, w) in enumerate(ch2):
            slab = wp.tile([P, n1t, w], F16, name=f"w2s{i}", tag=f"w2s{i}")
            ring_of[("w2", i)].dma_start(out=slab[:], in_=w2pk[:, pos:pos + n1t * w])
            w2s.append(slab)
            pos += n1t * w

        # PE warmup: dummy matmuls so the HAM clock gate reaches 8/8
        # before real work arrives (transpose-mode does not count).
        psw = pacc.tile([P, P], F32, tag="ps2")
        for i in range(18):
            nc.tensor.matmul(psw[:], lhsT=ident[:], rhs=ident[:],
                             start=True, stop=True)

        h1 = act.tile([P, n1], F16, tag="h1")
        h1Tb = act.tile([P, n1t, P], F16, tag="h1Tb")
        h2s = [act.tile([P, P], F16, name=f"h2_{i}", tag=f"h2_{i}")
               for i in range(3)]
        h2Tb = act.tile([P, n2t, P], F16, tag="h2Tb")
        ps3 = ptr.tile([NOUT, P], F32, tag="ps3")
        pts = [ptr.tile([P, P], F16, name=f"pt{i}", tag=f"pt{i}") for i in range(3)]
        o = act.tile([NOUT, P], F32, tag="o")
        pti = 0

        # Deferred per-chunk epilogues.  Emitting chunk i's transposes
        # AFTER chunk i+1's matmuls keeps the PE's strict in-order queue
        # from head-of-line blocking on DVE/ACT relu+copy latency.
        def l1_epilogue(i):
            nonlocal pti
            c0, w = ch1[i]
            # relu for the whole chunk (DVE/ACT alternating)
            if i % 2 == 0:
                nc.vector.tensor_scalar_max(h1[:, c0:c0 + w], ps1[i][:], 0.0)
            else:
                nc.scalar.activation(h1[:, c0:c0 + w], ps1[i][:],
                                     mybir.ActivationFunctionType.Relu)

        def l1_transposes(i):
            nonlocal pti
            c0, w = ch1[i]
            for j in range(w // P):
                jg = (c0 // P) + j
                pt = pts[pti % 3]; pti += 1
                nc.tensor.transpose(pt[:], h1[:, jg * P:(jg + 1) * P], ident[:])
                if pti % 2 == 0:
                    nc.scalar.copy(h1Tb[:, jg, :], pt[:])
                else:
                    nc.vector.tensor_copy(h1Tb[:, jg, :], pt[:])

        def l2_epilogue(i):
            nonlocal pti
            c0, w = ch2[i]
            for j in range(w // P):
                jg = (c0 // P) + j
                pt = pts[pti % 3]
                h2 = h2s[pti % 3]
                pti += 1
                if pti % 2 == 0:
                    nc.vector.tensor_scalar_max(h2[:], ps2[i][:, j * P:(j + 1) * P], 0.0)
                else:
                    nc.scalar.activation(h2[:], ps2[i][:, j * P:(j + 1) * P],
                                         mybir.ActivationFunctionType.Relu)
                nc.tensor.transpose(pt[:], h2[:], ident[:])
                if pti % 2 == 0:
                    nc.scalar.copy(h2Tb[:, jg, :], pt[:])
                else:
                    nc.vector.tensor_copy(h2Tb[:, jg, :], pt[:])
                nc.tensor.matmul(ps3[:], lhsT=w3t[:, jg, :], rhs=h2Tb[:, jg, :],
                                 start=(jg == 0), stop=(jg == n2t - 1))

        # ---- Layer 1, software-pipelined by one chunk
        ps1 = [pacc.tile([P, w], F32, name=f"ps1_{i}", tag=f"ps{i % 2}")
               for i, (_, w) in enumerate(ch1)]
        for i in range(len(ch1)):
            for t in range(KT1):
                nc.tensor.matmul(ps1[i][:], lhsT=xts[t], rhs=w1s[i][:, t, :],
                                 start=(t == 0), stop=(t == KT1 - 1))
            l1_epilogue(i)
            if i >= 1:
                l1_transposes(i - 1)
        l1_transposes(len(ch1) - 1)

        # ---- Layer 2, software-pipelined by one chunk (h2 transposes +
        # per-tile L3 accumulation trail by one chunk)
        ps2 = [pacc.tile([P, w], F32, name=f"ps2_{i}", tag=f"ps{2 + i % 2}")
               for i, (_, w) in enumerate(ch2)]
        for i in range(len(ch2)):
            for t in range(n1t):
                nc.tensor.matmul(ps2[i][:], lhsT=h1Tb[:, t, :], rhs=w2s[i][:, t, :],
                                 start=(t == 0), stop=(t == n1t - 1))
            if i >= 1:
                l2_epilogue(i - 1)
        l2_epilogue(len(ch2) - 1)

        nc.vector.tensor_add(o[:], ps3[:], b3t[:].to_broadcast([NOUT, P]))
        nc.sync.dma_start(out=out, in_=o[:])


def _get_program(n1t, n2t):
    key = (n1t, n2t)
    if key in _CACHE:
        return _CACHE[key]
    nc = bacc.Bacc("TRN2", target_bir_lowering=False, debug=False,
                   enable_asserts=False, enable_partition_id=False,
                   num_devices=NCORES)
    n1, n2 = n1t * P, n2t * P
    xT = nc.dram_tensor("xT", [P, KT1, BS], F16, kind="ExternalInput").ap()
    w1pk = nc.dram_tensor("w1pk", [P, KT1 * n1], F16, kind="ExternalInput").ap()
    w2pk = nc.dram_tensor("w2pk", [P, n1t * n2], F16, kind="ExternalInput").ap()
    w3p = nc.dram_tensor("w3p", [P, n2t, NOUT], F16, kind="ExternalInput").ap()
    b3r = nc.dram_tensor("b3r", [NOUT, 1], F32, kind="ExternalInput").ap()
    out = nc.dram_tensor("out", [NOUT, BS], F32, kind="ExternalOutput").ap()
    with tile.TileContext(nc) as tc:
        _mlp_body(tc, n1t, n2t, xT, w1pk, w2pk, w3p, b3r, out)
    nc.compile()
    _CACHE[key] = nc
    return nc


def kernel(x, w1, idx1, w2, idx2, masks1, masks2, W3, b3, context):
    global LAST_RESULT
    x = np.ascontiguousarray(np.asarray(x, dtype=np.float32))
    ctxi = int(np.asarray(context))

    weff1 = _build_weff(np.asarray(w1), np.asarray(idx1),
                        np.asarray(masks1)[ctxi], NIN)
    weff2 = _build_weff(np.asarray(w2), np.asarray(idx2),
                        np.asarray(masks2)[ctxi], NH)

    # Exact dead-unit pruning: units whose Weff row is identically zero
    # output relu(0)=0 and contribute nothing downstream.
    a1 = np.flatnonzero(np.abs(weff1).sum(1))
    a2 = np.flatnonzero(np.abs(weff2).sum(1))
    n1t = max(1, -(-len(a1) // P))
    n2t = max(1, -(-len(a2) // P))
    n1, n2 = n1t * P, n2t * P

    w1p = np.zeros((n1, NIN), np.float32); w1p[:len(a1)] = weff1[a1]
    w2p = np.zeros((n2, n1), np.float32)
    w2p[:len(a2), :len(a1)] = weff2[np.ix_(a2, a1)]
    W3p = np.zeros((n2, NOUT), np.float32); W3p[:len(a2)] = np.asarray(W3)[a2]

    # w1 transposed to (n_in, n1), rows zero-padded to 7*128
    w1t = np.zeros((KT1 * P, n1), np.float32); w1t[:NIN] = w1p.T
    w2t = w2p.T                                            # (n1, n2)

    w1pk = _pack_chunks(w1t.astype(np.float16), KT1, _chunks(n1, 128))
    w2pk = _pack_chunks(w2t.astype(np.float16), n1t, _chunks(n2, 512))

    w3p = np.zeros((P, n2t, NOUT), np.float16)
    w3p[:, :, :] = W3p.astype(np.float16).reshape(n2t, P, NOUT).transpose(1, 0, 2)
    b3r = np.ascontiguousarray(
        np.asarray(b3, dtype=np.float32).reshape(NOUT, 1))

    try:
        import antenv.axon_hooks  # noqa: F401
    except Exception:
        os.environ.setdefault("BASS_NEVER_TRACE", "1")

    nc = _get_program(n1t, n2t)
    in_maps = []
    for c in range(NCORES):
        xs = x[c * BS:(c + 1) * BS].T.astype(np.float16)   # (784, 128)
        xT = np.zeros((P, KT1, BS), np.float16)
        for t in range(KT1):
            rows = xs[t * P:(t + 1) * P]
            xT[:rows.shape[0], t, :] = rows
        in_maps.append({"xT": xT, "w1pk": w1pk, "w2pk": w2pk, "w3p": w3p,
                        "b3r": b3r})

    LAST_RESULT = run_bass_kernel_spmd(nc, in_maps, list(range(NCORES)))
    return np.concatenate(
        [LAST_RESULT.results[c]["out"].T for c in range(NCORES)], axis=0)
